# revision 5
# baseline (speedup 1.0000x reference)
"""Trainium2 Bass kernel for nn_LstmDecoder (B=1024, T=256, F=256, D=128).

Structure:
- The reference's "reverse" pass reverses the FEATURE dim (not time), so with
  host-side weight/init permutations both passes become plain LSTMs over the
  same input X. Masking (out=0 where t >= n_im) and the tail copy
  (shapes/gap/slope) are applied host-side during output assembly.
- Data parallel over batch: core c takes rows c::8 (interleaved so ragged
  lengths stay balanced).
- Device layout is fully transposed ("orientation B"): feature/hidden on
  partitions, batch on the free dim. Host pre-transposes inputs and
  un-transposes outputs, so the device does zero transposes.
- Per time step, gates^T accumulate in PSUM: 2 feature chunks (K=128) +
  tail chunk (K=5: shapes, gap, slope, bias-one) as float32r matmuls batched
  over 2 time steps (N=256), plus the recurrent h-part (K=128, N=128) per
  step in float32. Elementwise: sigmoid/tanh on ScalarE, muls/adds on
  VectorE, output staging copy on GpSimd.
"""

import os
import sys

import numpy as np

for _p in ("/opt/trn_rl_repo", "/root/.axon_site/_ro/trn_rl_repo"):
    if os.path.isdir(_p) and _p not in sys.path:
        sys.path.append(_p)

import concourse.bass as bass
import concourse.bacc as bacc
import concourse.mybir as mybir
import concourse.tile as tile

F32 = mybir.dt.float32
F32R = mybir.dt.float32r
AF = mybir.ActivationFunctionType

B, T, F, D = 1024, 256, 256, 128
NCORES = 8
BL = B // NCORES  # 128 batch rows per core


# ----------------------------------------------------------------------------
# Host-side weight preparation
# ----------------------------------------------------------------------------

def _prep_weights(W_ih, W_hh, b_ih, b_hh, init_h, init_c, init_h_rev, init_c_rev):
    """Per-pass (Wih(512,260), Whh(512,128), b(512), h0(128), c0(128)) with
    gate order [i,f,o,g]; pass2 input-permuted (feature reversal) and
    output-permuted (hidden reversal)."""

    def reorder(M):  # torch gate order i,f,g,o -> i,f,o,g
        return np.concatenate([M[0:D], M[D:2*D], M[3*D:4*D], M[2*D:3*D]], axis=0)

    b = (b_ih + b_hh).astype(np.float32)
    p1 = (reorder(W_ih), reorder(W_hh), reorder(b[:, None])[:, 0],
          init_h[0].astype(np.float32), init_c[0].astype(np.float32))

    perm_in = np.concatenate([np.arange(F)[::-1], [F + 1, F], [F + 2], [F + 3]])
    rows = np.concatenate([g * D + (D - 1 - np.arange(D)) for g in range(4)])
    Wih2 = W_ih[:, perm_in][rows]
    Whh2 = W_hh[rows][:, ::-1]
    b2 = b[rows]
    p2 = (reorder(Wih2), reorder(Whh2), reorder(b2[:, None])[:, 0],
          init_h_rev[0, ::-1].astype(np.float32),
          init_c_rev[0, ::-1].astype(np.float32))
    return p1, p2


# ----------------------------------------------------------------------------
# Device program builder
# ----------------------------------------------------------------------------

def build_nc(t_steps=T, group=16):
    """Build the Bass program. Returns nc."""
    assert t_steps % group == 0 and group % 2 == 0
    nc = bacc.Bacc("TRN2")

    x0 = nc.declare_dram_parameter("x0", [128, t_steps, BL], F32, isOutput=False)
    x1 = nc.declare_dram_parameter("x1", [128, t_steps, BL], F32, isOutput=False)
    xt = nc.declare_dram_parameter("xt", [5, t_steps, BL], F32, isOutput=False)
    wih0 = nc.declare_dram_parameter("wih0", [2, 128, 512], F32, isOutput=False)
    wih1 = nc.declare_dram_parameter("wih1", [2, 128, 512], F32, isOutput=False)
    wtail = nc.declare_dram_parameter("wtail", [2, 5, 512], F32, isOutput=False)
    whh = nc.declare_dram_parameter("whh", [2, 128, 512], F32, isOutput=False)
    hc = nc.declare_dram_parameter("hc", [2, 2, 128, BL], F32, isOutput=False)
    outs = [
        nc.declare_dram_parameter("out1", [128, t_steps, BL], F32, isOutput=True),
        nc.declare_dram_parameter("out2", [128, t_steps, BL], F32, isOutput=True),
    ]

    with tile.TileContext(nc) as tc:
        with (
            tc.tile_pool(name="weights", bufs=1) as wp,
            tc.tile_pool(name="state", bufs=1) as sp,
            tc.tile_pool(name="xin", bufs=2) as xp,
            tc.tile_pool(name="ostg", bufs=2) as op,
            tc.tile_pool(name="psum", bufs=2, space="PSUM") as pp,
        ):
            W0, W1, WT, WH, HT, GC, S, TC, TMP = [], [], [], [], [], [], [], [], []
            for p in range(2):
                W0.append(wp.tile([128, 512], F32R, tag=f"w0_{p}", name=f"w0_{p}"))
                W1.append(wp.tile([128, 512], F32R, tag=f"w1_{p}", name=f"w1_{p}"))
                WT.append(wp.tile([5, 512], F32R, tag=f"wt_{p}", name=f"wt_{p}"))
                WH.append(wp.tile([128, 512], F32, tag=f"wh_{p}", name=f"wh_{p}"))
                nc.gpsimd.dma_start(out=W0[p], in_=wih0[p])
                nc.gpsimd.dma_start(out=W1[p], in_=wih1[p])
                nc.gpsimd.dma_start(out=WT[p], in_=wtail[p])
                nc.sync.dma_start(out=WH[p], in_=whh[p])
                HT.append(sp.tile([128, BL], F32, tag=f"h_{p}", name=f"h_{p}"))
                GC.append(sp.tile([128, 2 * BL], F32, tag=f"gc_{p}", name=f"gc_{p}"))
                S.append(sp.tile([128, 3 * BL], F32, tag=f"s_{p}", name=f"s_{p}"))
                TC.append(sp.tile([128, BL], F32, tag=f"tc_{p}", name=f"tc_{p}"))
                TMP.append(sp.tile([128, 2 * BL], F32, tag=f"tmp_{p}", name=f"tmp_{p}"))
                nc.sync.dma_start(out=HT[p], in_=hc[p, 0])
                nc.sync.dma_start(out=GC[p][:, BL:2 * BL], in_=hc[p, 1])

            ngroups = t_steps // group
            for gi in range(ngroups):
                t0 = gi * group
                X0 = xp.tile([128, group, BL], F32R, tag="x0")
                X1 = xp.tile([128, group, BL], F32R, tag="x1")
                XT = xp.tile([5, group, BL], F32R, tag="xt")
                nc.gpsimd.dma_start(out=X0, in_=x0[:, t0:t0 + group, :])
                nc.gpsimd.dma_start(out=X1, in_=x1[:, t0:t0 + group, :])
                nc.gpsimd.dma_start(out=XT, in_=xt[:, t0:t0 + group, :])
                O = [op.tile([128, group, BL], F32, tag="o1", name="o1"),
                     op.tile([128, group, BL], F32, tag="o2", name="o2")]

                for si in range(group // 2):
                    for p in range(2):
                        # psum: [gate, t_in_pair, b]; gates 0,1 -> bank A,
                        # gates 2,3 -> bank B of this 2-bank tile
                        ps = pp.tile([128, 4, 2, BL], F32, tag=f"ps_{p}", name=f"ps_{p}")
                        # x-part: 3 chunks x 4 gates, N=2*BL, float32r
                        for g in range(4):
                            for ci, (Wc, Xc) in enumerate(
                                ((W0[p], X0), (W1[p], X1), (WT[p], XT))
                            ):
                                nc.tensor.matmul(
                                    ps[:, g, :, :],
                                    lhsT=Wc[:, g * 128:(g + 1) * 128],
                                    rhs=Xc[:, 2 * si:2 * si + 2, :],
                                    start=(ci == 0 and g % 2 == 0),
                                    stop=False,
                                    skip_group_check=True,
                                )
                        for s in range(2):
                            t = t0 + 2 * si + s
                            # recurrent part: 4 gates, N=BL, float32
                            for g in range(4):
                                nc.tensor.matmul(
                                    ps[:, g, s, :],
                                    lhsT=WH[p][:, g * 128:(g + 1) * 128],
                                    rhs=HT[p],
                                    start=False,
                                    stop=(s == 1 and g % 2 == 1),
                                    skip_group_check=True,
                                )
                            # sigmoid over gates i,f,o
                            nc.scalar.activation(
                                out=S[p], in_=ps[:, 0:3, s, :], func=AF.Sigmoid)
                            # tanh over gate g
                            nc.scalar.activation(
                                out=GC[p][:, 0:BL], in_=ps[:, 3, s, :], func=AF.Tanh)
                            # [i*g~ | f*c]
                            nc.vector.tensor_mul(
                                TMP[p], S[p][:, 0:2 * BL], GC[p])
                            # c' = i*g~ + f*c
                            nc.vector.tensor_add(
                                GC[p][:, BL:2 * BL],
                                TMP[p][:, 0:BL], TMP[p][:, BL:2 * BL])
                            nc.scalar.activation(
                                out=TC[p], in_=GC[p][:, BL:2 * BL], func=AF.Tanh)
                            # h = o * tanh(c)
                            nc.vector.tensor_mul(
                                HT[p], S[p][:, 2 * BL:3 * BL], TC[p])
                            nc.gpsimd.tensor_copy(
                                out=O[p][:, 2 * si + s, :], in_=HT[p])

                for p in range(2):
                    nc.sync.dma_start(out=outs[p][:, t0:t0 + group, :], in_=O[p])
    nc.finalize()
    return nc


# ----------------------------------------------------------------------------
# Host prep / run / assemble
# ----------------------------------------------------------------------------

def _host_prep(inputs, t_steps=T):
    enc = np.asarray(inputs["encoded_feature"], np.float32)
    shp = np.asarray(inputs["shapes"], np.float32)
    gap = np.asarray(inputs["gap_means"], np.float32)
    slp = np.asarray(inputs["slopes"], np.float32)
    p1, p2 = _prep_weights(
        np.asarray(inputs["W_ih"], np.float32), np.asarray(inputs["W_hh"], np.float32),
        np.asarray(inputs["b_ih"], np.float32), np.asarray(inputs["b_hh"], np.float32),
        np.asarray(inputs["init_h"], np.float32), np.asarray(inputs["init_c"], np.float32),
        np.asarray(inputs["init_h_rev"], np.float32),
        np.asarray(inputs["init_c_rev"], np.float32))

    wih0 = np.stack([np.ascontiguousarray(p[0][:, 0:128].T) for p in (p1, p2)])
    wih1 = np.stack([np.ascontiguousarray(p[0][:, 128:256].T) for p in (p1, p2)])
    wtail = np.stack([
        np.concatenate([p[0][:, 256:260].T, p[2][None, :]], axis=0)
        for p in (p1, p2)]).astype(np.float32)
    whhT = np.stack([np.ascontiguousarray(p[1].T) for p in (p1, p2)])
    hc = np.stack([
        np.stack([np.repeat(p[3][:, None], BL, 1), np.repeat(p[4][:, None], BL, 1)])
        for p in (p1, p2)]).astype(np.float32)

    in_maps = []
    for c in range(NCORES):
        enc_c = enc[c::NCORES, :t_steps]          # (BL, t, 256)
        x0 = np.ascontiguousarray(enc_c[:, :, 0:128].transpose(2, 1, 0))
        x1 = np.ascontiguousarray(enc_c[:, :, 128:256].transpose(2, 1, 0))
        xt = np.empty((5, t_steps, BL), np.float32)
        xt[0] = shp[c::NCORES, :t_steps, 0].T
        xt[1] = shp[c::NCORES, :t_steps, 1].T
        xt[2] = gap[c::NCORES][None, :]
        xt[3] = slp[c::NCORES][None, :]
        xt[4] = 1.0
        in_maps.append({
            "x0": x0, "x1": x1, "xt": xt,
            "wih0": wih0, "wih1": wih1, "wtail": wtail, "whh": whhT, "hc": hc,
        })
    return in_maps


def _assemble(inputs, results, t_steps=T):
    shp = np.asarray(inputs["shapes"], np.float32)
    gap = np.asarray(inputs["gap_means"], np.float32)
    slp = np.asarray(inputs["slopes"], np.float32)
    n = np.asarray(inputs["n_ims"]).astype(np.int64)
    out = np.empty((B, t_steps, 2 * D + 4), np.float32)
    for c in range(NCORES):
        out[c::NCORES, :, 0:D] = results[c]["out1"].transpose(2, 1, 0)
        out[c::NCORES, :, D:2 * D] = results[c]["out2"].transpose(2, 1, 0)
    mask = (np.arange(t_steps)[None, :] < n[:, None]).astype(np.float32)
    out[:, :, 0:2 * D] *= mask[:, :, None]
    out[:, :, 2 * D:2 * D + 2] = shp[:, :t_steps]
    out[:, :, 2 * D + 2] = gap[:, None]
    out[:, :, 2 * D + 3] = slp[:, None]
    return out


_NC_CACHE = {}


def _get_nc(t_steps=T, group=16):
    key = (t_steps, group)
    if key not in _NC_CACHE:
        _NC_CACHE[key] = build_nc(t_steps, group)
    return _NC_CACHE[key]


def run_on_device(inputs, t_steps=T, group=16, trace=False):
    from concourse.bass_utils import run_bass_kernel_spmd
    nc = _get_nc(t_steps, group)
    in_maps = _host_prep(inputs, t_steps)
    res = run_bass_kernel_spmd(
        nc, in_maps, core_ids=list(range(NCORES)), trace=trace)
    return res


def kernel(**inputs):
    res = run_on_device(inputs)
    return _assemble(inputs, res.results)


# revision 9
# speedup vs baseline: 1.4310x; 1.4310x over previous
"""Trainium2 Bass kernel for nn_LstmDecoder (B=1024, T=256, F=256, D=128).

Structure:
- The reference's "reverse" pass reverses the FEATURE dim (not time), so with
  host-side weight/init permutations both passes become plain LSTMs over the
  same input X. Masking (out=0 where t >= n_im) and the tail copy
  (shapes/gap/slope) are applied host-side during output assembly.
- Data parallel over batch: core c takes rows c::8 (interleaved so the
  descending-sorted ragged lengths stay balanced across cores).
- Device layout is fully transposed ("orientation B"): feature/hidden on
  partitions, batch on the free dim. Host pre-transposes inputs and
  un-transposes outputs, so the device does zero transposes.
- Ragged specialization: the program is traced for the actual n_ims. At step
  t only nb(t) batch columns are active (lengths sorted descending), so every
  matmul / activation / elementwise op is sliced to the active width, and
  time steps are grouped into adaptive blocks so x-part matmuls keep a free
  dim near 256.
- All matmul operands are float16 (1 cycle/row on the PE; fp32 is 4x slower
  and fp32r lowers to 2-pass fp32-HIGH). PSUM accumulation stays fp32 and all
  elementwise math (sigmoid/tanh on ScalarE, mul/add on VectorE) is fp32; only
  the h state is rounded to fp16 (~5e-4) to feed the recurrent matmul.
"""

import os
import sys

import numpy as np

for _p in ("/opt/trn_rl_repo", "/root/.axon_site/_ro/trn_rl_repo"):
    if os.path.isdir(_p) and _p not in sys.path:
        sys.path.append(_p)

import concourse.bass as bass
import concourse.bacc as bacc
import concourse.mybir as mybir
import concourse.tile as tile

F32 = mybir.dt.float32
F16 = mybir.dt.float16
AF = mybir.ActivationFunctionType

B, T, F, D = 1024, 256, 256, 128
NCORES = 8
BL = B // NCORES  # 128 batch rows per core
GROUP = 16        # time steps per DMA/staging group


# ----------------------------------------------------------------------------
# Host-side weight preparation
# ----------------------------------------------------------------------------

def _prep_weights(W_ih, W_hh, b_ih, b_hh, init_h, init_c, init_h_rev, init_c_rev):
    """Per-pass (Wih(512,260), Whh(512,128), b(512), h0(128), c0(128)) with
    gate order [i,f,o,g]; pass2 input-permuted (feature reversal) and
    output-permuted (hidden reversal)."""

    def reorder(M):  # torch gate order i,f,g,o -> i,f,o,g
        return np.concatenate([M[0:D], M[D:2*D], M[3*D:4*D], M[2*D:3*D]], axis=0)

    b = (b_ih + b_hh).astype(np.float32)
    p1 = (reorder(W_ih), reorder(W_hh), reorder(b[:, None])[:, 0],
          init_h[0].astype(np.float32), init_c[0].astype(np.float32))

    perm_in = np.concatenate([np.arange(F)[::-1], [F + 1, F], [F + 2], [F + 3]])
    rows = np.concatenate([g * D + (D - 1 - np.arange(D)) for g in range(4)])
    Wih2 = W_ih[:, perm_in][rows]
    Whh2 = W_hh[rows][:, ::-1]
    b2 = b[rows]
    p2 = (reorder(Wih2), reorder(Whh2), reorder(b2[:, None])[:, 0],
          init_h_rev[0, ::-1].astype(np.float32),
          init_c_rev[0, ::-1].astype(np.float32))
    return p1, p2


def _nb_schedule(n_ims, t_steps):
    """Active batch width per step, max over cores (interleaved sharding),
    rounded up to a multiple of 8 and clamped to [8, BL]."""
    n = np.asarray(n_ims).astype(np.int64)
    nb = []
    for t in range(t_steps):
        cnt = int((n > t).sum())
        w = -(-cnt // NCORES)          # ceil
        w = min(BL, max(8, 8 * -(-w // 8)))
        nb.append(w)
    return tuple(nb)


def _block_schedule(nb, t_steps):
    """Adaptive time blocks: consecutive steps sharing one 2-bank psum tile.
    Block at t0 takes P steps with P * nb[t0] <= 256, within one GROUP."""
    blocks = []
    t = 0
    while t < t_steps:
        p = max(1, 256 // nb[t])
        p = min(p, GROUP - (t % GROUP), t_steps - t)
        blocks.append((t, p))
        t += p
    return blocks


# ----------------------------------------------------------------------------
# Device program builder
# ----------------------------------------------------------------------------

def build_nc(t_steps=T, nb=None):
    if nb is None:
        nb = (BL,) * t_steps
    blocks = _block_schedule(nb, t_steps)
    nc = bacc.Bacc("TRN2")

    x0 = nc.declare_dram_parameter("x0", [128, t_steps, BL], F16, isOutput=False)
    x1 = nc.declare_dram_parameter("x1", [128, t_steps, BL], F16, isOutput=False)
    xt = nc.declare_dram_parameter("xt", [5, t_steps, BL], F16, isOutput=False)
    wih0 = nc.declare_dram_parameter("wih0", [2, 128, 512], F16, isOutput=False)
    wih1 = nc.declare_dram_parameter("wih1", [2, 128, 512], F16, isOutput=False)
    wtail = nc.declare_dram_parameter("wtail", [2, 5, 512], F16, isOutput=False)
    whh = nc.declare_dram_parameter("whh", [2, 128, 512], F16, isOutput=False)
    h0d = nc.declare_dram_parameter("h0d", [2, 128, BL], F16, isOutput=False)
    c0d = nc.declare_dram_parameter("c0d", [2, 128, BL], F32, isOutput=False)
    outs = [
        nc.declare_dram_parameter("out1", [128, t_steps, BL], F32, isOutput=True),
        nc.declare_dram_parameter("out2", [128, t_steps, BL], F32, isOutput=True),
    ]

    with tile.TileContext(nc) as tc:
        with (
            tc.tile_pool(name="weights", bufs=1) as wp,
            tc.tile_pool(name="state", bufs=1) as sp,
            tc.tile_pool(name="xin", bufs=2) as xp,
            tc.tile_pool(name="ostg", bufs=2) as op,
            tc.tile_pool(name="psum", bufs=2, space="PSUM") as pp,
        ):
            W0, W1, WT, WH, HT, GC, S, TC, TMP = [], [], [], [], [], [], [], [], []
            for p in range(2):
                W0.append(wp.tile([128, 512], F16, tag=f"w0_{p}", name=f"w0_{p}"))
                W1.append(wp.tile([128, 512], F16, tag=f"w1_{p}", name=f"w1_{p}"))
                WT.append(wp.tile([5, 512], F16, tag=f"wt_{p}", name=f"wt_{p}"))
                WH.append(wp.tile([128, 512], F16, tag=f"wh_{p}", name=f"wh_{p}"))
                nc.sync.dma_start(out=W0[p], in_=wih0[p])
                nc.sync.dma_start(out=W1[p], in_=wih1[p])
                nc.sync.dma_start(out=WT[p], in_=wtail[p])
                nc.sync.dma_start(out=WH[p], in_=whh[p])
                HT.append(sp.tile([128, BL], F16, tag=f"h_{p}", name=f"h_{p}"))
                GC.append(sp.tile([128, 2 * BL], F32, tag=f"gc_{p}", name=f"gc_{p}"))
                S.append(sp.tile([128, 3 * BL], F32, tag=f"s_{p}", name=f"s_{p}"))
                TC.append(sp.tile([128, BL], F32, tag=f"tc_{p}", name=f"tc_{p}"))
                TMP.append(sp.tile([128, 2 * BL], F32, tag=f"tmp_{p}", name=f"tmp_{p}"))
                nc.sync.dma_start(out=HT[p], in_=h0d[p])
                nc.sync.dma_start(out=GC[p][:, BL:2 * BL], in_=c0d[p])

            ngroups = t_steps // GROUP
            for gi in range(ngroups):
                t0 = gi * GROUP
                w = nb[t0]  # max active width in this group (nb non-increasing)
                X0 = xp.tile([128, GROUP, BL], F16, tag="x0", name="x0t")
                X1 = xp.tile([128, GROUP, BL], F16, tag="x1", name="x1t")
                XT = xp.tile([5, GROUP, BL], F16, tag="xt", name="xtt")
                nc.sync.dma_start(out=X0[:, :, 0:w], in_=x0[:, t0:t0 + GROUP, 0:w])
                nc.sync.dma_start(out=X1[:, :, 0:w], in_=x1[:, t0:t0 + GROUP, 0:w])
                nc.sync.dma_start(out=XT[:, :, 0:w], in_=xt[:, t0:t0 + GROUP, 0:w])
                O = [op.tile([128, GROUP, BL], F32, tag="o1", name="o1"),
                     op.tile([128, GROUP, BL], F32, tag="o2", name="o2")]

                for (tb, P) in [blk for blk in blocks if t0 <= blk[0] < t0 + GROUP]:
                    wb = nb[tb]
                    for p in range(2):
                        # psum tile: gate g occupies slot [g, 0:256] (half a
                        # bank, never straddles banks); steps packed at s*wb
                        ps = pp.tile([128, 4, 256], F32, tag=f"ps_{p}",
                                     name=f"ps_{p}")
                        # x-part: 3 chunks x 4 gates, N = P*wb, fp16
                        for g in range(4):
                            for ci, (Wc, Xc) in enumerate(
                                ((W0[p], X0), (W1[p], X1), (WT[p], XT))
                            ):
                                nc.tensor.matmul(
                                    ps[:, g, 0:P * wb],
                                    lhsT=Wc[:, g * 128:(g + 1) * 128],
                                    rhs=Xc[:, tb - t0:tb - t0 + P, 0:wb],
                                    start=(ci == 0 and g % 2 == 0),
                                    stop=False,
                                    skip_group_check=True,
                                )
                        for s in range(P):
                            t = tb + s
                            a = nb[t]
                            # recurrent part: 4 gates, N = a, fp16
                            for g in range(4):
                                nc.tensor.matmul(
                                    ps[:, g, s * wb:s * wb + a],
                                    lhsT=WH[p][:, g * 128:(g + 1) * 128],
                                    rhs=HT[p][:, 0:a],
                                    start=False,
                                    stop=(s == P - 1 and g % 2 == 1),
                                    skip_group_check=True,
                                )
                            # sigmoid over gates i,f,o
                            nc.scalar.activation(
                                out=S[p].rearrange("k (x b) -> k x b", x=3)[:, :, 0:a],
                                in_=ps[:, 0:3, s * wb:s * wb + a], func=AF.Sigmoid)
                            # tanh over gate g
                            nc.scalar.activation(
                                out=GC[p][:, 0:a], in_=ps[:, 3, s * wb:s * wb + a],
                                func=AF.Tanh)
                            # [i*g~ | f*c]
                            nc.vector.tensor_mul(
                                TMP[p].rearrange("k (x b) -> k x b", x=2)[:, :, 0:a],
                                S[p].rearrange("k (x b) -> k x b", x=3)[:, 0:2, 0:a],
                                GC[p].rearrange("k (x b) -> k x b", x=2)[:, :, 0:a])
                            # c' = i*g~ + f*c
                            nc.vector.tensor_add(
                                GC[p][:, BL:BL + a],
                                TMP[p][:, 0:a], TMP[p][:, BL:BL + a])
                            nc.scalar.activation(
                                out=TC[p][:, 0:a], in_=GC[p][:, BL:BL + a],
                                func=AF.Tanh)
                            # h = o * tanh(c), rounded to fp16 for the
                            # recurrent matmul
                            nc.vector.tensor_mul(
                                HT[p][:, 0:a], S[p][:, 2 * BL:2 * BL + a],
                                TC[p][:, 0:a])
                            nc.gpsimd.tensor_copy(
                                out=O[p][:, t - t0, 0:w], in_=HT[p][:, 0:w])

                for p in range(2):
                    nc.sync.dma_start(
                        out=outs[p][:, t0:t0 + GROUP, 0:w],
                        in_=O[p][:, :, 0:w])
    nc.finalize()
    return nc


# ----------------------------------------------------------------------------
# Host prep / run / assemble
# ----------------------------------------------------------------------------

def _host_prep(inputs, t_steps=T):
    enc = np.asarray(inputs["encoded_feature"], np.float32)
    shp = np.asarray(inputs["shapes"], np.float32)
    gap = np.asarray(inputs["gap_means"], np.float32)
    slp = np.asarray(inputs["slopes"], np.float32)
    p1, p2 = _prep_weights(
        np.asarray(inputs["W_ih"], np.float32), np.asarray(inputs["W_hh"], np.float32),
        np.asarray(inputs["b_ih"], np.float32), np.asarray(inputs["b_hh"], np.float32),
        np.asarray(inputs["init_h"], np.float32), np.asarray(inputs["init_c"], np.float32),
        np.asarray(inputs["init_h_rev"], np.float32),
        np.asarray(inputs["init_c_rev"], np.float32))

    wih0 = np.stack([np.ascontiguousarray(p[0][:, 0:128].T) for p in (p1, p2)]).astype(np.float16)
    wih1 = np.stack([np.ascontiguousarray(p[0][:, 128:256].T) for p in (p1, p2)]).astype(np.float16)
    wtail = np.stack([
        np.concatenate([p[0][:, 256:260].T, p[2][None, :]], axis=0)
        for p in (p1, p2)]).astype(np.float16)
    whhT = np.stack([np.ascontiguousarray(p[1].T) for p in (p1, p2)]).astype(np.float16)
    h0d = np.stack([np.repeat(p[3][:, None], BL, 1) for p in (p1, p2)]).astype(np.float16)
    c0d = np.stack([np.repeat(p[4][:, None], BL, 1) for p in (p1, p2)]).astype(np.float32)

    in_maps = []
    for c in range(NCORES):
        enc_c = enc[c::NCORES, :t_steps]          # (BL, t, 256)
        x0 = np.ascontiguousarray(
            enc_c[:, :, 0:128].transpose(2, 1, 0)).astype(np.float16)
        x1 = np.ascontiguousarray(
            enc_c[:, :, 128:256].transpose(2, 1, 0)).astype(np.float16)
        xtl = np.empty((5, t_steps, BL), np.float16)
        xtl[0] = shp[c::NCORES, :t_steps, 0].T
        xtl[1] = shp[c::NCORES, :t_steps, 1].T
        xtl[2] = gap[c::NCORES][None, :]
        xtl[3] = slp[c::NCORES][None, :]
        xtl[4] = 1.0
        in_maps.append({
            "x0": x0, "x1": x1, "xt": xtl,
            "wih0": wih0, "wih1": wih1, "wtail": wtail, "whh": whhT,
            "h0d": h0d, "c0d": c0d,
        })
    return in_maps


def _assemble(inputs, results, t_steps=T):
    shp = np.asarray(inputs["shapes"], np.float32)
    gap = np.asarray(inputs["gap_means"], np.float32)
    slp = np.asarray(inputs["slopes"], np.float32)
    n = np.asarray(inputs["n_ims"]).astype(np.int64)
    out = np.empty((B, t_steps, 2 * D + 4), np.float32)
    for c in range(NCORES):
        out[c::NCORES, :, 0:D] = results[c]["out1"].transpose(2, 1, 0)
        out[c::NCORES, :, D:2 * D] = results[c]["out2"].transpose(2, 1, 0)
    mask = (np.arange(t_steps)[None, :] < n[:, None])[:, :, None]
    out[:, :, 0:2 * D] = np.where(mask, out[:, :, 0:2 * D], 0.0)
    out[:, :, 2 * D:2 * D + 2] = shp[:, :t_steps]
    out[:, :, 2 * D + 2] = gap[:, None]
    out[:, :, 2 * D + 3] = slp[:, None]
    return out


_NC_CACHE = {}


def _get_nc(t_steps, nb):
    key = (t_steps, nb)
    if key not in _NC_CACHE:
        _NC_CACHE[key] = build_nc(t_steps, nb)
    return _NC_CACHE[key]


def run_on_device(inputs, t_steps=T, trace=False):
    from concourse.bass_utils import run_bass_kernel_spmd
    nb = _nb_schedule(inputs["n_ims"], t_steps)
    nc = _get_nc(t_steps, nb)
    in_maps = _host_prep(inputs, t_steps)
    res = run_bass_kernel_spmd(
        nc, in_maps, core_ids=list(range(NCORES)), trace=trace)
    return res


def kernel(**inputs):
    res = run_on_device(inputs)
    return _assemble(inputs, res.results)


# revision 12
# speedup vs baseline: 1.7040x; 1.1907x over previous
"""Trainium2 Bass kernel for nn_LstmDecoder (B=1024, T=256, F=256, D=128).

Structure:
- The reference's "reverse" pass reverses the FEATURE dim (not time), so with
  host-side weight/init permutations both passes become plain LSTMs over the
  same input X. Masking (out=0 where t >= n_im) and the tail copy
  (shapes/gap/slope) are applied host-side during output assembly.
- Data parallel over batch: core c takes rows c::8 (interleaved so the
  descending-sorted ragged lengths stay balanced across cores).
- Device layout is fully transposed ("orientation B"): feature/hidden on
  partitions, batch on the free dim. Host pre-transposes inputs and
  un-transposes outputs, so the device does zero transposes.
- Ragged specialization: the program is traced for the actual n_ims. At step
  t only nb(t) batch columns are active (lengths sorted descending), so every
  matmul / activation / elementwise op is sliced to the active width, and
  time steps are grouped into adaptive blocks so x-part matmuls keep a free
  dim near 256.
- All matmul operands are float16 (1 cycle/row on the PE; fp32 is 4x slower
  and fp32r lowers to 2-pass fp32-HIGH). PSUM accumulation stays fp32 and all
  elementwise math (sigmoid/tanh on ScalarE, mul/add on VectorE) is fp32; only
  the h state is rounded to fp16 (~5e-4) to feed the recurrent matmul.
"""

import os
import sys

import numpy as np

for _p in ("/opt/trn_rl_repo", "/root/.axon_site/_ro/trn_rl_repo"):
    if os.path.isdir(_p) and _p not in sys.path:
        sys.path.append(_p)

import concourse.bass as bass
import concourse.bacc as bacc
import concourse.mybir as mybir
import concourse.tile as tile

F32 = mybir.dt.float32
F16 = mybir.dt.float16
AF = mybir.ActivationFunctionType

B, T, F, D = 1024, 256, 256, 128
NCORES = 8
BL = B // NCORES  # 128 batch rows per core
GROUP = 16        # time steps per DMA/staging group


# ----------------------------------------------------------------------------
# Host-side weight preparation
# ----------------------------------------------------------------------------

def _prep_weights(W_ih, W_hh, b_ih, b_hh, init_h, init_c, init_h_rev, init_c_rev):
    """Per-pass (Wih(512,260), Whh(512,128), b(512), h0(128), c0(128)) with
    gate order [i,f,o,g]; pass2 input-permuted (feature reversal) and
    output-permuted (hidden reversal)."""

    def reorder(M):  # torch gate order i,f,g,o -> i,f,o,g
        return np.concatenate([M[0:D], M[D:2*D], M[3*D:4*D], M[2*D:3*D]], axis=0)

    b = (b_ih + b_hh).astype(np.float32)
    p1 = (reorder(W_ih), reorder(W_hh), reorder(b[:, None])[:, 0],
          init_h[0].astype(np.float32), init_c[0].astype(np.float32))

    perm_in = np.concatenate([np.arange(F)[::-1], [F + 1, F], [F + 2], [F + 3]])
    rows = np.concatenate([g * D + (D - 1 - np.arange(D)) for g in range(4)])
    Wih2 = W_ih[:, perm_in][rows]
    Whh2 = W_hh[rows][:, ::-1]
    b2 = b[rows]
    p2 = (reorder(Wih2), reorder(Whh2), reorder(b2[:, None])[:, 0],
          init_h_rev[0, ::-1].astype(np.float32),
          init_c_rev[0, ::-1].astype(np.float32))
    return p1, p2


def _nb_schedule(n_ims, t_steps):
    """Active batch width per step, max over cores (interleaved sharding),
    rounded up to a multiple of 8 and clamped to [8, BL]."""
    n = np.asarray(n_ims).astype(np.int64)
    nb = []
    for t in range(t_steps):
        cnt = int((n > t).sum())
        w = -(-cnt // NCORES)          # ceil
        w = min(BL, max(8, 8 * -(-w // 8)))
        nb.append(w)
    return tuple(nb)


def _block_schedule(nb, t_steps):
    """Adaptive time blocks: consecutive steps sharing one 2-bank psum tile.
    Block at t0 takes P steps with P * nb[t0] <= 256, within one GROUP."""
    blocks = []
    t = 0
    while t < t_steps:
        p = max(1, 256 // nb[t])
        p = min(p, GROUP - (t % GROUP), t_steps - t)
        blocks.append((t, p))
        t += p
    return blocks


# ----------------------------------------------------------------------------
# Device program builder
# ----------------------------------------------------------------------------

def build_nc(t_steps=T, nb=None):
    if nb is None:
        nb = (BL,) * t_steps
    blocks = _block_schedule(nb, t_steps)
    nc = bacc.Bacc("TRN2")

    x0 = nc.declare_dram_parameter("x0", [128, t_steps, BL], F16, isOutput=False)
    x1 = nc.declare_dram_parameter("x1", [128, t_steps, BL], F16, isOutput=False)
    xt = nc.declare_dram_parameter("xt", [5, t_steps, BL], F16, isOutput=False)
    wih0 = nc.declare_dram_parameter("wih0", [2, 128, 512], F16, isOutput=False)
    wih1 = nc.declare_dram_parameter("wih1", [2, 128, 512], F16, isOutput=False)
    wtail = nc.declare_dram_parameter("wtail", [2, 128, 512], F16, isOutput=False)
    whh = nc.declare_dram_parameter("whh", [2, 128, 512], F16, isOutput=False)
    h0d = nc.declare_dram_parameter("h0d", [2, 128, BL], F16, isOutput=False)
    c0d = nc.declare_dram_parameter("c0d", [2, 128, BL], F32, isOutput=False)
    outs = [
        nc.declare_dram_parameter("out1", [128, t_steps, BL], F16, isOutput=True),
        nc.declare_dram_parameter("out2", [128, t_steps, BL], F16, isOutput=True),
    ]

    with tile.TileContext(nc) as tc:
        with (
            tc.tile_pool(name="weights", bufs=1) as wp,
            tc.tile_pool(name="state", bufs=1) as sp,
            tc.tile_pool(name="xin", bufs=2) as xp,
            tc.tile_pool(name="ostg", bufs=2) as op,
            tc.tile_pool(name="psum", bufs=2, space="PSUM") as pp,
        ):
            W0, W1, WT, WH, HT, GC, S, TC, TMP = [], [], [], [], [], [], [], [], []
            for p in range(2):
                W0.append(wp.tile([128, 512], F16, tag=f"w0_{p}", name=f"w0_{p}"))
                W1.append(wp.tile([128, 512], F16, tag=f"w1_{p}", name=f"w1_{p}"))
                WT.append(wp.tile([128, 512], F16, tag=f"wt_{p}", name=f"wt_{p}"))
                WH.append(wp.tile([128, 512], F16, tag=f"wh_{p}", name=f"wh_{p}"))
                nc.sync.dma_start(out=W0[p], in_=wih0[p])
                nc.sync.dma_start(out=W1[p], in_=wih1[p])
                nc.sync.dma_start(out=WT[p], in_=wtail[p])
                nc.sync.dma_start(out=WH[p], in_=whh[p])
                HT.append(sp.tile([128, BL], F16, tag=f"h_{p}", name=f"h_{p}"))
                GC.append(sp.tile([128, 2 * BL], F32, tag=f"gc_{p}", name=f"gc_{p}"))
                S.append(sp.tile([128, 3 * BL], F32, tag=f"s_{p}", name=f"s_{p}"))
                TC.append(sp.tile([128, BL], F32, tag=f"tc_{p}", name=f"tc_{p}"))
                TMP.append(sp.tile([128, 2 * BL], F32, tag=f"tmp_{p}", name=f"tmp_{p}"))
                nc.sync.dma_start(out=HT[p], in_=h0d[p])
                nc.sync.dma_start(out=GC[p][:, BL:2 * BL], in_=c0d[p])

            ngroups = t_steps // GROUP
            O = [None, None]
            for gi in range(ngroups):
                t0 = gi * GROUP
                w = nb[t0]  # max active width in this group (nb non-increasing)
                X0 = xp.tile([128, GROUP, BL], F16, tag="x0", name="x0t")
                X1 = xp.tile([128, GROUP, BL], F16, tag="x1", name="x1t")
                XT = xp.tile([128, GROUP, BL], F16, tag="xt", name="xtt")
                nc.sync.dma_start(out=X0[:, :, 0:w], in_=x0[:, t0:t0 + GROUP, 0:w])
                nc.sync.dma_start(out=X1[:, :, 0:w], in_=x1[:, t0:t0 + GROUP, 0:w])
                if gi < 2:
                    # rows 5: of the tail chunk have zero weights; zero the
                    # whole tile once per buffer so they stay finite
                    # (NaN*0 hazard) and initialized for the simulator
                    nc.gpsimd.memset(XT, 0.0)
                nc.sync.dma_start(out=XT[0:5, :, 0:w], in_=xt[:, t0:t0 + GROUP, 0:w])
                Oprev = O
                O = [op.tile([128, GROUP, BL], F16, tag="o1", name="o1"),
                     op.tile([128, GROUP, BL], F16, tag="o2", name="o2")]

                for (tb, P) in [blk for blk in blocks if t0 <= blk[0] < t0 + GROUP]:
                    wb = nb[tb]
                    for p in range(2):
                        # psum tile: gate g occupies slot [g, 0:256] (half a
                        # bank, never straddles banks); steps packed at s*wb
                        ps = pp.tile([128, 4, 256], F32, tag=f"ps_{p}",
                                     name=f"ps_{p}")
                        # x-part: 3 chunks x 4 gates, N = P*wb, fp16
                        for g in range(4):
                            for ci, (Wc, Xc) in enumerate(
                                ((W0[p], X0), (W1[p], X1), (WT[p], XT))
                            ):
                                nc.tensor.matmul(
                                    ps[:, g, 0:P * wb],
                                    lhsT=Wc[:, g * 128:(g + 1) * 128],
                                    rhs=Xc[:, tb - t0:tb - t0 + P, 0:wb],
                                    start=(ci == 0 and g % 2 == 0),
                                    stop=False,
                                    skip_group_check=True,
                                )
                        for s in range(P):
                            t = tb + s
                            a = nb[t]
                            # recurrent part: 4 gates, N = a, fp16
                            if t == 0:
                                hsrc = HT[p][:, 0:a]
                            elif t - t0 == 0:
                                hsrc = Oprev[p][:, GROUP - 1, 0:a]
                            else:
                                hsrc = O[p][:, t - t0 - 1, 0:a]
                            for g in range(4):
                                nc.tensor.matmul(
                                    ps[:, g, s * wb:s * wb + a],
                                    lhsT=WH[p][:, g * 128:(g + 1) * 128],
                                    rhs=hsrc,
                                    start=False,
                                    stop=(s == P - 1 and g % 2 == 1),
                                    skip_group_check=True,
                                )
                            # sigmoid over gates i,f,o
                            nc.scalar.activation(
                                out=S[p].rearrange("k (x b) -> k x b", x=3)[:, :, 0:a],
                                in_=ps[:, 0:3, s * wb:s * wb + a], func=AF.Sigmoid)
                            # tanh over gate g
                            nc.scalar.activation(
                                out=GC[p][:, 0:a], in_=ps[:, 3, s * wb:s * wb + a],
                                func=AF.Tanh)
                            # [i*g~ | f*c]
                            nc.vector.tensor_mul(
                                TMP[p].rearrange("k (x b) -> k x b", x=2)[:, :, 0:a],
                                S[p].rearrange("k (x b) -> k x b", x=3)[:, 0:2, 0:a],
                                GC[p].rearrange("k (x b) -> k x b", x=2)[:, :, 0:a])
                            # c' = i*g~ + f*c
                            nc.vector.tensor_add(
                                GC[p][:, BL:BL + a],
                                TMP[p][:, 0:a], TMP[p][:, BL:BL + a])
                            nc.scalar.activation(
                                out=TC[p][:, 0:a], in_=GC[p][:, BL:BL + a],
                                func=AF.Tanh)
                            # h = o * tanh(c), written fp16 straight into
                            # the output staging slot (also the recurrent
                            # matmul's input for step t+1)
                            nc.vector.tensor_mul(
                                O[p][:, t - t0, 0:w],
                                S[p][:, 2 * BL:2 * BL + w], TC[p][:, 0:w])

                for p in range(2):
                    nc.sync.dma_start(
                        out=outs[p][:, t0:t0 + GROUP, 0:w],
                        in_=O[p][:, :, 0:w])
    nc.finalize()
    return nc


# ----------------------------------------------------------------------------
# Host prep / run / assemble
# ----------------------------------------------------------------------------

def _host_prep(inputs, t_steps=T):
    enc = np.asarray(inputs["encoded_feature"], np.float32)
    shp = np.asarray(inputs["shapes"], np.float32)
    gap = np.asarray(inputs["gap_means"], np.float32)
    slp = np.asarray(inputs["slopes"], np.float32)
    p1, p2 = _prep_weights(
        np.asarray(inputs["W_ih"], np.float32), np.asarray(inputs["W_hh"], np.float32),
        np.asarray(inputs["b_ih"], np.float32), np.asarray(inputs["b_hh"], np.float32),
        np.asarray(inputs["init_h"], np.float32), np.asarray(inputs["init_c"], np.float32),
        np.asarray(inputs["init_h_rev"], np.float32),
        np.asarray(inputs["init_c_rev"], np.float32))

    wih0 = np.stack([np.ascontiguousarray(p[0][:, 0:128].T) for p in (p1, p2)]).astype(np.float16)
    wih1 = np.stack([np.ascontiguousarray(p[0][:, 128:256].T) for p in (p1, p2)]).astype(np.float16)
    wtail = np.zeros((2, 128, 512), np.float16)
    for i, p in enumerate((p1, p2)):
        wtail[i, 0:5] = np.concatenate(
            [p[0][:, 256:260].T, p[2][None, :]], axis=0).astype(np.float16)
    whhT = np.stack([np.ascontiguousarray(p[1].T) for p in (p1, p2)]).astype(np.float16)
    h0d = np.stack([np.repeat(p[3][:, None], BL, 1) for p in (p1, p2)]).astype(np.float16)
    c0d = np.stack([np.repeat(p[4][:, None], BL, 1) for p in (p1, p2)]).astype(np.float32)

    in_maps = []
    for c in range(NCORES):
        enc_c = enc[c::NCORES, :t_steps]          # (BL, t, 256)
        x0 = np.ascontiguousarray(
            enc_c[:, :, 0:128].transpose(2, 1, 0)).astype(np.float16)
        x1 = np.ascontiguousarray(
            enc_c[:, :, 128:256].transpose(2, 1, 0)).astype(np.float16)
        xtl = np.empty((5, t_steps, BL), np.float16)
        xtl[0] = shp[c::NCORES, :t_steps, 0].T
        xtl[1] = shp[c::NCORES, :t_steps, 1].T
        xtl[2] = gap[c::NCORES][None, :]
        xtl[3] = slp[c::NCORES][None, :]
        xtl[4] = 1.0
        in_maps.append({
            "x0": x0, "x1": x1, "xt": xtl,
            "wih0": wih0, "wih1": wih1, "wtail": wtail, "whh": whhT,
            "h0d": h0d, "c0d": c0d,
        })
    return in_maps


def _assemble(inputs, results, t_steps=T):
    shp = np.asarray(inputs["shapes"], np.float32)
    gap = np.asarray(inputs["gap_means"], np.float32)
    slp = np.asarray(inputs["slopes"], np.float32)
    n = np.asarray(inputs["n_ims"]).astype(np.int64)
    out = np.empty((B, t_steps, 2 * D + 4), np.float32)
    for c in range(NCORES):
        out[c::NCORES, :, 0:D] = results[c]["out1"].transpose(2, 1, 0)
        out[c::NCORES, :, D:2 * D] = results[c]["out2"].transpose(2, 1, 0)
    mask = (np.arange(t_steps)[None, :] < n[:, None])[:, :, None]
    out[:, :, 0:2 * D] = np.where(mask, out[:, :, 0:2 * D], 0.0)
    out[:, :, 2 * D:2 * D + 2] = shp[:, :t_steps]
    out[:, :, 2 * D + 2] = gap[:, None]
    out[:, :, 2 * D + 3] = slp[:, None]
    return out


_NC_CACHE = {}


def _get_nc(t_steps, nb):
    key = (t_steps, nb)
    if key not in _NC_CACHE:
        _NC_CACHE[key] = build_nc(t_steps, nb)
    return _NC_CACHE[key]


def run_on_device(inputs, t_steps=T, trace=False):
    from concourse.bass_utils import run_bass_kernel_spmd
    nb = _nb_schedule(inputs["n_ims"], t_steps)
    nc = _get_nc(t_steps, nb)
    in_maps = _host_prep(inputs, t_steps)
    res = run_bass_kernel_spmd(
        nc, in_maps, core_ids=list(range(NCORES)), trace=trace)
    return res


def kernel(**inputs):
    res = run_on_device(inputs)
    return _assemble(inputs, res.results)
